# revision 3
# baseline (speedup 1.0000x reference)
"""Distributed Bass kernel for gated-adapter attention (head-sharded TP), v2.

v2 vs v1: all device I/O is bf16 and sharded to minimize host<->device
traffic. x arrives as a per-core token slice and is AllGathered on-device;
wo^T arrives as a per-core head-row slice and is AllGathered on-device
(overlapping stages A/B). Debug probes removed; y returned bf16.

Layout (per core, n_local = H/n_cores heads):
  stage A: QKV projections from xg (device-AllGathered [D, token] blocks).
           Weights pre-transposed/permuted on host (even/odd per head for
           RoPE as contiguous half-tile DVE ops). Q^T/K^T stored
           [128(d), n_local, BT] bf16, V stored [128(tok), n_vt, d_local].
  stage B: causal attention, scores transposed (S^T[k, q]), no-max softmax,
           two-segment softmax with tanh(gate)-scaled adapter segment.
  stage C: AllToAll redistributes per-head outputs to per-token-chunk,
           then output projection with the AllGathered wo^T.
"""

import math
import numpy as np

from concourse import bass, bacc, tile
from concourse.tile_rust import add_dep_helper
from concourse import mybir

F32 = mybir.dt.float32
BF16 = mybir.dt.bfloat16
AF = mybir.ActivationFunctionType
OP = mybir.AluOpType

B, S, D, H, L = 2, 2048, 2048, 16, 10
HD = 128
N_CORES = 8


def build_attn(n_cores=8, TBLK=512, QBLK=512, stages="ABC"):
    n_local = H // n_cores
    d_local = n_local * HD
    NT = B * S              # real tokens
    BT = NT + L             # projected token-columns (adapter appended)
    TPC = NT // n_cores     # tokens per core for the output projection
    XCOLS = TBLK + 16       # per-core upload: TBLK own tokens + adapter pad
    nkt = D // 128          # contraction tiles for projections
    n_vt = (BT + 127) // 128
    n_jb = S // QBLK        # query blocks per batch
    ndiag = QBLK // 128     # diagonal k-tiles per query block
    nn = D // 512
    assert NT % (n_cores * TBLK) == 0 and QBLK % 128 == 0
    inv_sqrt_hd = 1.0 / math.sqrt(HD)
    groups = [list(range(n_cores))]

    nc = bacc.Bacc(None, num_devices=n_cores, debug=False)

    # xw packs per-core columns: [ xs | wqt | wkt | wvt ]
    xw = nc.declare_dram_parameter("xw", [D, XCOLS + 3 * d_local], BF16, False)
    wos = nc.declare_dram_parameter("wos", [d_local, D], BF16, False)
    # tabs: this core's column slice of the stacked [cos;sin] RoPE table
    SSH = S // n_cores
    tabs = nc.declare_dram_parameter("tabs", [128, SSH], BF16, False)
    gate2 = nc.declare_dram_parameter("gate2", [1, n_local], F32, False)
    y_out = nc.declare_dram_parameter("y", [TPC, D], BF16, True)

    # x-token blocks in global order; every core sources all of them from
    # the AllGathered xg (program must be identical across cores).
    jp_pairs = [tuple(j for j in (a, a + 1) if j < n_jb)
                for a in range(0, n_jb, 2)]

    with tile.TileContext(nc) as tc:
        with (
            tc.tile_pool(name="res", bufs=1) as res,
            tc.tile_pool(name="dram", bufs=1, space="DRAM") as dram,
        ):
            # ---- resident tensors ----
            qt_s = res.tile([128, n_local, BT], BF16)
            kt_s = res.tile([128, n_local, BT], BF16)
            v_s = res.tile([128, n_vt, d_local], BF16)
            cm_s = res.tile([128, QBLK + (ndiag - 1) * 128], BF16)
            ones128 = res.tile([128, 128], BF16)
            ones_l = res.tile([L, 128], BF16)
            avs = res.tile([L, d_local], BF16)

            nc.gpsimd.memset(ones128[:], 1.0)
            nc.gpsimd.memset(ones_l[:], 1.0)

            # ---- device-side gathers (start immediately, overlap stage A) --
            # collectives cannot read IO tensors: stage params into Internal
            # DRAM first.
            tbl = dram.tile([128, SSH], BF16, name="tbl")
            wol = dram.tile([d_local, D], BF16, name="wol")
            tbg = dram.tile([n_cores * 128, SSH], BF16, name="tbg",
                            addr_space="Shared")
            # column(token)-chunk split: chunk A = first HBLK tokens of every
            # core's slice, chunk B = the rest. A complete token block is
            # computable as soon as its chunk lands.
            HBLK = TBLK // 2
            XB = XCOLS - HBLK
            xga = dram.tile([n_cores * D, HBLK], BF16, name="xga",
                            addr_space="Shared")
            xgb = dram.tile([n_cores * D, XB], BF16, name="xgb",
                            addr_space="Shared")
            wog = dram.tile([D, D], BF16, name="wog", addr_space="Shared")
            # x gather split into token-chunks so the first token blocks can
            # be fully projected while the second chunk is still in flight.
            xla = dram.tile([D, HBLK], BF16, name="xla")
            xlb = dram.tile([D, XB], BF16, name="xlb")
            cpx1 = nc.sync.dma_start(xla.opt(), xw[:, 0:HBLK])
            cpx2 = nc.sync.dma_start(xlb.opt(), xw[:, HBLK:XCOLS])
            cpt = nc.gpsimd.dma_start(tbl.opt(), tabs[:, :])
            cpw = nc.scalar.dma_start(wol.opt(), wos[:, :])
            # tabs first (tiny, and RoPE gates stage A's PSUM pipeline)
            cc_t = nc.gpsimd.collective_compute(
                "AllGather", OP.bypass, replica_groups=groups,
                ins=[tbl.opt()], outs=[tbg.opt()])
            add_dep_helper(cc_t.ins, cpt.ins, sync=True,
                           reason="allgather waits for tabs staging")
            cc_x1 = nc.gpsimd.collective_compute(
                "AllGather", OP.bypass, replica_groups=groups,
                ins=[xla.opt()], outs=[xga.opt()])
            add_dep_helper(cc_x1.ins, cpx1.ins, sync=True,
                           reason="allgather waits for x staging")
            cc_x2 = nc.gpsimd.collective_compute(
                "AllGather", OP.bypass, replica_groups=groups,
                ins=[xlb.opt()], outs=[xgb.opt()])
            add_dep_helper(cc_x2.ins, cpx2.ins, sync=True,
                           reason="allgather waits for x staging")
            cc_w = nc.gpsimd.collective_compute(
                "AllGather", OP.bypass, replica_groups=groups,
                ins=[wol.opt()], outs=[wog.opt()])
            add_dep_helper(cc_w.ins, cpw.ins, sync=True,
                           reason="allgather waits for wo staging")

            TPC2 = S // n_cores
            otl_b = [dram.tile([n_cores * d_local, TPC2], BF16,
                               name=f"otl{b}") for b in range(B)]
            ogc_b = [dram.tile([n_cores * d_local, TPC2], BF16,
                               name=f"ogc{b}") for b in range(B)]

            # ---- weight prep (QKV bf16 into SBUF, from packed xw cols) ----
            wqp = tc.alloc_tile_pool(name="wqp", bufs=1)
            wq_r = wqp.tile([128, nkt, d_local], BF16)
            wk_r = wqp.tile([128, nkt, d_local], BF16)
            wv_r = wqp.tile([128, nkt, d_local], BF16)
            for wi, dst in enumerate((wq_r, wk_r, wv_r)):
                co = XCOLS + wi * d_local
                for kt in range(nkt):
                    eng = nc.scalar if kt % 2 == 0 else nc.gpsimd
                    eng.dma_start(dst[:, kt, :],
                                  xw[kt * 128:(kt + 1) * 128,
                                     co:co + d_local])
            # causal tile mask, synthesized on device:
            # cm_s[p, v] = 1.0 if v - (ndiag-1)*128 - p >= 0 else 0.0
            nc.gpsimd.memset(cm_s[:], 1.0)
            nc.gpsimd.affine_select(
                out=cm_s[:], in_=cm_s[:], compare_op=OP.is_ge, fill=0.0,
                base=-(ndiag - 1) * 128, channel_multiplier=-1,
                pattern=[[1, QBLK + (ndiag - 1) * 128]])

            # ---- stage A: QKV projections + RoPE ----
            with (
                tc.tile_pool(name="xp", bufs=3) as xp,
                tc.tile_pool(name="rp", bufs=2) as rp,
                tc.tile_pool(name="csp", bufs=1) as csp,
                tc.tile_pool(name="psA", bufs=5, space="PSUM") as psA,
                tc.tile_pool(name="psV", bufs=3, space="PSUM") as psV,
            ):
                # cs_a = [cos;sin], cs_b = [sin;cos] rebuilt from the
                # AllGathered per-core table slices.
                cs_a = csp.tile([128, S], BF16)
                cs_b = csp.tile([128, S], BF16)
                for r in range(n_cores):
                    eng = (nc.sync, nc.scalar)[r % 2]
                    la = eng.dma_start(cs_a[:, r * SSH:(r + 1) * SSH],
                                       tbg[r * 128:(r + 1) * 128, :])
                    lb1 = eng.dma_start(cs_b[0:64, r * SSH:(r + 1) * SSH],
                                        tbg[r * 128 + 64:(r + 1) * 128, :])
                    lb2 = eng.dma_start(cs_b[64:128, r * SSH:(r + 1) * SSH],
                                        tbg[r * 128:r * 128 + 64, :])
                    for ld in (la, lb1, lb2):
                        add_dep_helper(ld.ins, cc_t.ins, sync=True,
                                       reason="cs read waits tabs allgather")
                # adapter block first (local, no gather dep), then chunk-A
                # half-blocks for all ranks, then chunk-B half-blocks
                blocks = [(NT, L, None, 0)] + [
                    (r * TBLK + h * HBLK, HBLK, r, h)
                    for h in range(2) for r in range(NT // TBLK)]
                for (c0, w, r, h) in (blocks if "A" in stages else []):
                    is_adapter = r is None
                    xbf = xp.tile([128, nkt, HBLK], BF16, tag="xbf")
                    for kt in range(nkt):
                        eng = nc.sync if kt % 2 == 0 else nc.gpsimd
                        if is_adapter:
                            ld = eng.dma_start(
                                xbf[:, kt, :w],
                                xw[kt * 128:(kt + 1) * 128, TBLK:TBLK + w])
                        else:
                            # token-chunk gather layout: rank-major [D, HBLK]
                            src = xga if h == 0 else xgb
                            row = r * D + kt * 128
                            ld = eng.dma_start(
                                xbf[:, kt, :w],
                                src[row:row + 128, 0:w])
                            cc = cc_x1 if h == 0 else cc_x2
                            add_dep_helper(ld.ins, cc.ins, sync=True,
                                           reason="xg read waits allgather")

                    s0 = c0 % S
                    for wbf, dest in ((wq_r, qt_s), (wk_r, kt_s)):
                        for m in range(n_local):
                            ps = psA.tile([128, HBLK], F32, tag="qk")
                            for kt in range(nkt):
                                nc.tensor.matmul(
                                    ps[:, :w],
                                    lhsT=wbf[:, kt, m * 128:(m + 1) * 128],
                                    rhs=xbf[:, kt, :w],
                                    start=(kt == 0), stop=(kt == nkt - 1))
                            dcol = dest[:, m, c0:c0 + w]
                            if is_adapter:
                                nc.scalar.copy(dcol, ps[:, :w])
                            else:
                                # RoPE with permuted layout (even dims in
                                # partitions 0:64, odd in 64:128). Products
                                # as two full-width DVE ops; combines read
                                # same-base half-tiles (HW requires matching
                                # operand start partitions).
                                ca = cs_a[:, s0:s0 + w]
                                cb = cs_b[:, s0:s0 + w]
                                m1 = rp.tile([128, HBLK], BF16, tag="m1")
                                m2 = rp.tile([128, HBLK], BF16, tag="m2")
                                mb = rp.tile([128, HBLK], BF16, tag="mb")
                                nc.vector.tensor_tensor(
                                    m1[:, :w], ps[:, :w], ca[:, :], OP.mult)
                                nc.vector.tensor_tensor(
                                    m2[:, :w], ps[:, :w], cb[:, :], OP.mult)
                                nc.vector.tensor_copy(
                                    mb[0:64, :w], m1[64:128, :w])
                                nc.vector.tensor_copy(
                                    mb[64:128, :w], m2[0:64, :w])
                                nc.vector.tensor_tensor(
                                    dest[0:64, m, c0:c0 + w],
                                    m1[0:64, :w], mb[0:64, :w], OP.subtract)
                                nc.vector.tensor_tensor(
                                    dest[64:128, m, c0:c0 + w],
                                    m2[64:128, :w], mb[64:128, :w], OP.add)
                    for tt in range((w + 127) // 128):
                        pw = min(128, w - tt * 128)
                        pv = psV.tile([128, d_local], F32, tag="v")
                        for kt in range(nkt):
                            nc.tensor.matmul(
                                pv[:pw, :],
                                lhsT=xbf[:, kt, tt * 128:tt * 128 + pw],
                                rhs=wv_r[:, kt, :],
                                start=(kt == 0), stop=(kt == nkt - 1))
                        nc.scalar.copy(v_s[:pw, c0 // 128 + tt, :], pv[:pw, :])

            wqp.release()

            # ---- gate: tanh + adapter-V scaling ----
            with tc.tile_pool(name="gp", bufs=1) as gp:
                gsb = gp.tile([1, n_local], F32)
                nc.sync.dma_start(gsb[:], gate2[:, :])
                tgh = gp.tile([1, n_local], F32)
                nc.scalar.activation(tgh[:], gsb[:], AF.Tanh)
                att = NT // 128
                for h in range(n_local):
                    tb = gp.tile([L, 1], F32, tag="tghb")
                    nc.gpsimd.partition_broadcast(tb[:], tgh[0:1, h:h + 1])
                    nc.vector.tensor_scalar(
                        avs[:, h * 128:(h + 1) * 128],
                        v_s[0:L, att, h * 128:(h + 1) * 128],
                        tb[:], None, OP.mult)

            # ---- wo^T prefetch from gathered wog (overlaps stage B) ----
            wotp = tc.alloc_tile_pool(name="wotp", bufs=1)
            wot_bf = wotp.tile([128, nkt, D], BF16)
            for kt in range(nkt):
                ld = nc.gpsimd.dma_start(wot_bf[:, kt, :],
                                         wog[kt * 128:(kt + 1) * 128, :])
                add_dep_helper(ld.ins, cc_w.ins, sync=True,
                               reason="wog read waits allgather")

            # ---- stage B: attention ----
            cp = tc.alloc_tile_pool(name="cp", bufs=2)
            og_sb = [cp.tile([128, nkt, S // n_cores], BF16, bufs=1,
                             name=f"ogsb{b}") for b in range(B)]
            with (
                tc.tile_pool(name="ep", bufs=6) as ep,
                tc.tile_pool(name="sp", bufs=3) as sp,
                tc.tile_pool(name="psO", bufs=3, space="PSUM") as psO,
                tc.tile_pool(name="psN", bufs=2, space="PSUM") as psN,
                tc.tile_pool(name="psW", bufs=3, space="PSUM") as psW,
            ):
                for b in (range(B) if "B" in stages else []):
                    otl_writes = []
                    for h in range(n_local):
                        for jp in jp_pairs:
                            out_ps = {}
                            norm_ps = {}
                            maxkt = {}
                            for j in jp:
                                out_ps[j] = psO.tile([128, QBLK], F32,
                                                     tag="outp", name=f"outp{j}")
                                norm_ps[j] = psN.tile([128, QBLK], F32,
                                                      tag="normp", name=f"normp{j}")
                                maxkt[j] = (j + 1) * ndiag
                            for kt in range(max(maxkt.values())):
                                for j in jp:
                                    if kt >= maxkt[j]:
                                        continue
                                    qsl = qt_s[:, h, b * S + j * QBLK:
                                               b * S + (j + 1) * QBLK]
                                    sps = psW.tile([128, QBLK], F32, tag="w")
                                    nc.tensor.matmul(
                                        sps[:],
                                        lhsT=kt_s[:, h, b * S + kt * 128:
                                                  b * S + (kt + 1) * 128],
                                        rhs=qsl, start=True, stop=True)
                                    e = ep.tile([128, QBLK], BF16, tag="e")
                                    nc.scalar.activation(e[:], sps[:], AF.Exp,
                                                         scale=inv_sqrt_hd)
                                    dk = kt - j * ndiag
                                    if dk >= 0:
                                        st = (ndiag - 1 - dk) * 128
                                        nc.vector.tensor_tensor(
                                            e[:], e[:], cm_s[:, st:st + QBLK],
                                            OP.mult)
                                    nc.tensor.matmul(
                                        norm_ps[j][:], lhsT=ones128[:], rhs=e[:],
                                        start=(kt == 0), stop=(kt == maxkt[j] - 1))
                                    tt = (b * S) // 128 + kt
                                    nc.tensor.matmul(
                                        out_ps[j][:],
                                        lhsT=v_s[:, tt, h * 128:(h + 1) * 128],
                                        rhs=e[:],
                                        start=(kt == 0), stop=(kt == maxkt[j] - 1))
                            for j in jp:
                                qsl = qt_s[:, h, b * S + j * QBLK:
                                           b * S + (j + 1) * QBLK]
                                sa = psW.tile([L, QBLK], F32, tag="w")
                                nc.tensor.matmul(
                                    sa[:], lhsT=kt_s[:, h, NT:NT + L],
                                    rhs=qsl, start=True, stop=True)
                                ea = ep.tile([L, QBLK], BF16, tag="ea")
                                nc.scalar.activation(ea[:], sa[:], AF.Exp,
                                                     scale=inv_sqrt_hd)
                                na = psW.tile([128, QBLK], F32, tag="w")
                                nc.tensor.matmul(na[:], lhsT=ones_l[:],
                                                 rhs=ea[:], start=True, stop=True)
                                # 1/x = exp(-ln x) on ACT (Reciprocal LUT is
                                # blocked; DVE reciprocal too slow for this)
                                lna = sp.tile([L, QBLK], F32, tag="lna")
                                nc.scalar.activation(lna[:], na[0:L, :], AF.Ln)
                                rca = sp.tile([L, QBLK], F32, tag="rca")
                                nc.scalar.activation(rca[:], lna[:], AF.Exp,
                                                     scale=-1.0)
                                pa = ep.tile([L, QBLK], BF16, tag="ea")
                                nc.vector.tensor_tensor(pa[:], ea[:], rca[:],
                                                        OP.mult)
                                oa = psW.tile([128, QBLK], F32, tag="w")
                                nc.tensor.matmul(
                                    oa[:], lhsT=avs[:, h * 128:(h + 1) * 128],
                                    rhs=pa[:], start=True, stop=True)
                                rcl = sp.tile([128, QBLK], F32, tag="rcl")
                                if j % 2 == 0:
                                    nc.vector.reciprocal(rcl[:],
                                                         norm_ps[j][:])
                                else:
                                    # 1/x = exp(-ln x); keeps the two j's of
                                    # a pair off the same engine
                                    lnn = sp.tile([128, QBLK], F32, tag="lnn")
                                    nc.scalar.activation(lnn[:],
                                                         norm_ps[j][:], AF.Ln)
                                    nc.scalar.activation(rcl[:], lnn[:],
                                                         AF.Exp, scale=-1.0)
                                tt1 = sp.tile([128, QBLK], F32, tag="t1c")
                                nc.vector.tensor_tensor(tt1[:], out_ps[j][:],
                                                        rcl[:], OP.mult)
                                fin = sp.tile([128, QBLK], BF16, tag="fin")
                                nc.vector.tensor_tensor(fin[:], tt1[:], oa[:],
                                                        OP.add)
                                for sc in range(QBLK // TPC2):
                                    scol = j * QBLK + sc * TPC2
                                    cidx = scol // TPC2
                                    wi = nc.sync.dma_start(
                                        otl_b[b][cidx * d_local + h * 128:
                                                 cidx * d_local + (h + 1) * 128,
                                                 :],
                                        fin[:, sc * TPC2:(sc + 1) * TPC2])
                                    otl_writes.append(wi)
                    # per-batch AllToAll + og prefetch, overlaps next batch
                    cc = nc.gpsimd.collective_compute(
                        "AllToAll", OP.bypass,
                        replica_groups=groups,
                        ins=[otl_b[b].opt()], outs=[ogc_b[b].opt()])
                    for wi in otl_writes:
                        add_dep_helper(cc.ins, wi.ins, sync=True,
                                       reason="a2a waits for otl writes")
                    for kt in range(nkt):
                        eng = nc.gpsimd if kt % 2 == 0 else nc.sync
                        ld = eng.dma_start(og_sb[b][:, kt, :],
                                           ogc_b[b][kt * 128:(kt + 1) * 128, :])
                        add_dep_helper(ld.ins, cc.ins, sync=True,
                                       reason="og load waits for a2a")

            # ---- stage C: output projection per batch ----
            ntc2 = TPC2 // 128
            with tc.tile_pool(name="psY", bufs=4, space="PSUM") as psY:
                for b in (range(B) if "C" in stages else []):
                    for tc_i in range(ntc2):
                        pys = [psY.tile([128, 512], F32, tag="y",
                                        name=f"py{b}_{tc_i}_{n}")
                               for n in range(nn)]
                        for kt in range(nkt):
                            for n in range(nn):
                                nc.tensor.matmul(
                                    pys[n][:],
                                    lhsT=og_sb[b][:, kt,
                                                  tc_i * 128:(tc_i + 1) * 128],
                                    rhs=wot_bf[:, kt, n * 512:(n + 1) * 512],
                                    start=(kt == 0), stop=(kt == nkt - 1))
                        for n in range(nn):
                            ysb = cp.tile([128, 512], BF16, tag="ysb")
                            nc.scalar.copy(ysb[:], pys[n][:])
                            nc.sync.dma_start(
                                y_out[b * TPC2 + tc_i * 128:
                                      b * TPC2 + (tc_i + 1) * 128,
                                      n * 512:(n + 1) * 512],
                                ysb[:])
            cp.release()
            wotp.release()

    nc.compile()
    return nc


def make_in_maps(x, adapter, wq, wk, wv, wo, gate, freqs_cis,
                 n_cores=8, TBLK=512, QBLK=512):
    """Host-side sharding/layout prep. Returns list of per-core input dicts."""
    import ml_dtypes
    bf16 = ml_dtypes.bfloat16
    n_local = H // n_cores
    ndiag = QBLK // 128
    NT = B * S
    XCOLS = TBLK + 16

    x = np.asarray(x, np.float32)
    adapter = np.asarray(adapter, np.float32)
    NT = B * S
    SSH = S // n_cores
    d_local = n_local * HD

    # x^T columns in global token order + adapter columns (replicated)
    xt = np.concatenate([x[b].T for b in range(B)], axis=1).astype(bf16)
    xa = np.zeros((D, 16), bf16)
    xa[:, :L] = adapter[0].T.astype(bf16)

    fc = np.asarray(freqs_cis, np.float32)
    cos = np.ascontiguousarray(fc[:, :, 0].T)
    sin = np.ascontiguousarray(fc[:, :, 1].T)
    csa = np.concatenate([cos, sin], axis=0).astype(bf16)   # [128, S]

    ev = np.arange(0, HD, 2)
    od = np.arange(1, HD, 2)
    head_perm = np.concatenate([ev, od])

    gate_f = np.asarray(gate, np.float32).reshape(H)
    wo_t = np.ascontiguousarray(np.asarray(wo, np.float32).T).astype(bf16)
    in_maps = []
    for c in range(n_cores):
        rows = []
        for hl in range(n_local):
            hg = c * n_local + hl
            rows.append(hg * HD + head_perm)
        rows_p = np.concatenate(rows)
        rows_n = np.arange(c * d_local, (c + 1) * d_local)
        xwp = np.empty((D, XCOLS + 3 * d_local), bf16)
        xwp[:, :TBLK] = xt[:, c * TBLK:(c + 1) * TBLK]
        xwp[:, TBLK:XCOLS] = xa
        xwp[:, XCOLS:XCOLS + d_local] = wq[rows_p, :].T.astype(bf16)
        xwp[:, XCOLS + d_local:XCOLS + 2 * d_local] = wk[rows_p, :].T.astype(bf16)
        xwp[:, XCOLS + 2 * d_local:] = wv[rows_n, :].T.astype(bf16)
        in_maps.append({
            "xw": xwp,
            "wos": wo_t[rows_n, :],
            "tabs": np.ascontiguousarray(csa[:, c * SSH:(c + 1) * SSH]),
            "gate2": gate_f[c * n_local:(c + 1) * n_local].reshape(1, n_local),
        })
    return in_maps


def assemble_output(results, n_cores=8):
    TPC2 = S // n_cores
    y = np.zeros((B, S, D), np.float32)
    for c in range(n_cores):
        yc = np.asarray(results[c]["y"], dtype=np.float32)
        for b in range(B):
            y[b, TPC2 * c:TPC2 * (c + 1), :] = yc[b * TPC2:(b + 1) * TPC2]
    return y


# ---------------------------------------------------------------------------
# Harness entry point: takes FULL inputs, returns FULL output.
# ---------------------------------------------------------------------------

_CACHE = {}


def kernel(x, adapter, wq, wk, wv, wo, gate, freqs_cis, mask):
    """Gated-adapter attention on 8 TRN2 NeuronCores (head-sharded TP)."""
    from concourse.bass_utils import run_bass_kernel_spmd

    nc = _CACHE.get("nc")
    if nc is None:
        nc = _CACHE["nc"] = build_attn(n_cores=N_CORES)
    in_maps = make_in_maps(x, adapter, wq, wk, wv, wo, gate, freqs_cis,
                           n_cores=N_CORES)
    r = run_bass_kernel_spmd(nc, in_maps, core_ids=list(range(N_CORES)))
    return assemble_output(r.results, n_cores=N_CORES)
